# revision 4
# baseline (speedup 1.0000x reference)
"""Self-contained trn2 Bass kernel v2: LSTM (B=64, T=512, I=128, H=1024,
forget_bias=1.0, tf gate order i,j,f,o) + per-timestep dense layer.

Strategy: gate/hidden sharding (each core owns 128 hidden units) with the
batch split into TWO independent 32-col streams that run phase-offset, so
one stream's activation/cell tail overlaps the other stream's matmuls.

All activations are SIGMOID: tanh(j) = 2*sigmoid(2j)-1 with the 2x folded
into the j-columns of Wx/Wh/b on the host, and tanh(c) = 2*sigmoid(2c)-1
via activation scale=2.0. The affine corrections fold into fused
scalar_tensor_tensor ops and a global 2x pre-scale of Wh/Wd (h' = h/2 is
what gets broadcast; exact in bf16). The forget bias (+1) rides in a
rank-1 bias matmul. Per (stream, step): 4 gate tiles [128,32] pack into
ONE PSUM bank (f|i|j|o along columns) so sigmoid(f,i) and sigmoid(j,o)
are single wide instructions; the cell update is 3 fused DVE ops; h' is
computed on the gpsimd engine right before trigger_dma (no semaphore hop
into the broadcast). PSUM banks are double-buffered by step parity so
next-step Wx matmuls need no extra synchronization.
"""

from contextlib import ExitStack

import numpy as np
import ml_dtypes

import concourse.bass as bass
import concourse.bacc as bacc
import concourse.mybir as mybir
from concourse.alu_op_type import AluOpType
from concourse.bass_utils import run_bass_kernel_spmd

F32 = mybir.dt.float32
BF16 = mybir.dt.bfloat16
AF = mybir.ActivationFunctionType

N_CORES = 8
B = 64
SB = 32                    # batch cols per stream
H = 1024
HC = H // N_CORES          # hidden units per core
G = 4 * HC                 # gate cols per core (tiles f,i,j,o)
PSUM_BANK = 512            # f32 elems per psum bank


def build(T, dense_blk, include_bias, include_dense_bias,
          act_order="fij_o", tc_slot="own", pe_wx="inline", dve_hp="early"):
    assert T % dense_blk == 0 and T % 2 == 0
    n_blk = T // dense_blk
    MD = dense_blk * 8

    nc = bacc.Bacc(target_bir_lowering=False)

    xt_d = nc.declare_dram_parameter("XT", [128, T * B], BF16, isOutput=False)
    wx_d = nc.declare_dram_parameter("WX", [128, G], BF16, isOutput=False)
    wh_d = nc.declare_dram_parameter("WH", [H, G], BF16, isOutput=False)
    wd_d = nc.declare_dram_parameter("WD", [H, 128], BF16, isOutput=False)
    brow_d = nc.declare_dram_parameter("BROW", [1, G], BF16, isOutput=False)
    if include_dense_bias:
        bd_d = nc.declare_dram_parameter("BD", [1, 128], BF16, isOutput=False)
    out_d = nc.declare_dram_parameter("OUT", [T * 8, 128], F32, isOutput=True)

    n_wdma = 4 + (1 if include_dense_bias else 0)
    n_consts = 1 + (1 if include_dense_bias else 0)
    bias_tiles = (0, 1, 2, 3) if include_bias else (0,)

    with ExitStack() as ctx:
        block = ctx.enter_context(nc.Block())
        sem = lambda n: ctx.enter_context(nc.semaphore(n))
        sb = lambda n, shape, dt: ctx.enter_context(nc.sbuf_tensor(n, shape, dt))

        wsem, xtsem, constsem, prep = (
            sem("wsem"), sem("xtsem"), sem("constsem"), sem("prep"))
        lsems = [sem(f"lsem{s}") for s in range(2)]
        histsem, densesem, outcp, outdma = (
            sem("histsem"), sem("densesem"), sem("outcp"), sem("outdma"))
        gsem = [sem(f"gsem{s}") for s in range(2)]
        asem = [sem(f"asem{s}") for s in range(2)]
        csem = [sem(f"csem{s}") for s in range(2)]
        a2sem = [sem(f"a2sem{s}") for s in range(2)]
        vsem, psem = sem("vsem"), sem("psem")
        hsems = [[[sem(f"hsem{k}_{s}_{q}") for q in range(2)] for s in range(2)]
                 for k in range(N_CORES)]

        xt_sb = sb("xt_sb", [128, T * B], BF16)
        wx_sb = sb("wx_sb", [128, G], BF16)
        wh_sb = sb("wh_sb", [128, 8 * G], BF16)
        wd_sb = sb("wd_sb", [128, 8 * 128], BF16)
        brow_sb = sb("brow_sb", [1, G], BF16)
        ones_sb = sb("ones_sb", [1, B], BF16)
        if include_dense_bias:
            bd_sb = sb("bd_sb", [1, 128], BF16)
            onesd_sb = sb("onesd_sb", [1, 128], BF16)
        sq = sb("sq", [128, 4 * 128], F32)          # (s*2+p)*128 | f i j o x32
        c_sb = sb("c_sb", [128, B], F32)            # stream s at s*SB
        cf_sb = sb("cf_sb", [128, B], F32)
        v_sb = sb("v_sb", [128, B], F32)
        tcb = sb("tcb", [128, B], F32)
        hstage = sb("hstage", [128, 4 * SB], BF16)  # (s*2+p)*SB
        hbuf = sb("hbuf", [128, 4 * 8 * SB], BF16)  # ((s*2+p)*8+slot)*SB
        hist = sb("hist", [128, 2 * 8 * dense_blk * 8], BF16)
        ostage = sb("ostage", [128, 2 * 128], F32)
        # 8 psum banks: tile (s, m) lives at the head of bank s*4+m (one
        # accumulation group per bank — multi-region groups break on HW).
        # The dense accumulator borrows the tail of bank 0.
        gates_ps = ctx.enter_context(nc.psum_tensor("gates_ps", [128, 8 * PSUM_BANK], F32))
        dense_ps = gates_ps[:, 128: 256]
        gates_r = gates_ps[:, :].rearrange("p (b v) -> p b v", v=PSUM_BANK)

        hist_cols = 8 * dense_blk * 8

        def gtile(s, m):
            base = (s * 4 + m) * PSUM_BANK
            return gates_ps[:, base: base + SB]

        def sq_cols(s, p, lo, hi):
            base = (s * 2 + p) * 128
            return sq[:, base + lo: base + hi]

        def hst(s, p):
            return hstage[:, (s * 2 + p) * SB: (s * 2 + p + 1) * SB]

        def hslot(s, p, k):
            base = ((s * 2 + p) * 8 + k) * SB
            return hbuf[:, base: base + SB]

        def nbc(tau):
            # broadcasts of parity tau%2 among steps 0..tau
            return tau // 2 + 1

        @block.sync
        def _(s: bass.BassEngine):
            s.dma_start(out=wx_sb[:, :], in_=wx_d[:, :]).then_inc(wsem, 16)
            s.dma_start(
                out=wh_sb[:, :].rearrange("p (c g) -> p c g", c=8),
                in_=wh_d[:, :].rearrange("(c p) g -> p c g", p=128),
            ).then_inc(wsem, 16)
            s.dma_start(
                out=wd_sb[:, :].rearrange("p (c o) -> p c o", c=8),
                in_=wd_d[:, :].rearrange("(c p) o -> p c o", p=128),
            ).then_inc(wsem, 16)
            s.dma_start(out=brow_sb[:, :], in_=brow_d[:, :]).then_inc(wsem, 16)
            if include_dense_bias:
                s.dma_start(out=bd_sb[:, :], in_=bd_d[:, :]).then_inc(wsem, 16)
            s.dma_start(out=xt_sb[:, :], in_=xt_d[:, :]).then_inc(xtsem, 16)
            for blk in range(n_blk):
                s.wait_ge(outcp, blk + 1)
                if blk >= 1:
                    s.wait_ge(outdma, 16 * blk)
                s.dma_start(
                    out=out_d[blk * MD: (blk + 1) * MD, :],
                    in_=ostage[:MD, (blk % 2) * 128: (blk % 2) * 128 + 128],
                ).then_inc(outdma, 16)
            s.wait_ge(outdma, 16 * n_blk)

        n_asem_step = 1 if act_order == "sig4" else 2

        @block.tensor
        def _(e: bass.BassTensorEngine):
            e.wait_ge(wsem, 16 * n_wdma)
            e.wait_ge(constsem, n_consts)
            e.wait_ge(xtsem, 16)

            def wx_group(s, t, final):
                # Per-tile accumulation groups: each tile owns a full psum
                # bank; start on its Wx matmul, stop on its last matmul.
                if t >= 1:
                    e.wait_ge(asem[s], n_asem_step * (t - 1) + n_asem_step)
                for m in range(4):
                    has_bias = m in bias_tiles
                    mm = e.matmul(
                        gtile(s, m),
                        lhsT=wx_sb[:, m * 128: (m + 1) * 128],
                        rhs=xt_sb[:, t * B + s * SB: t * B + s * SB + SB],
                        start=True,
                        stop=(final and not has_bias),
                        skip_group_check=True,
                    )
                    if has_bias:
                        mm = e.matmul(
                            gtile(s, m),
                            lhsT=brow_sb[0:1, m * 128: (m + 1) * 128],
                            rhs=ones_sb[0:1, 0:SB],
                            start=False,
                            stop=final,
                            skip_group_check=True,
                        )
                    if final and m in (2, 3):
                        mm.then_inc(gsem[s], 1)

            def dense_block(bi, t):
                bp = bi % 2
                e.wait_ge(histsem, dense_blk * (bi + 1))
                if bi >= 1:
                    e.wait_ge(outcp, bi)
                # bank 0 hosts the dense region: keep the f-tile read
                # (sigma_fij of stream A, step t) strictly before it
                e.wait_ge(asem[0], n_asem_step * t + 1)
                n_mm = 8 + (1 if include_dense_bias else 0)
                k = 0
                for c in range(8):
                    mm = e.matmul(
                        gates_ps[:MD, 128: 256],
                        lhsT=hist[:, bp * hist_cols + c * dense_blk * 8:][
                            :, : dense_blk * 8
                        ],
                        rhs=wd_sb[:, c * 128: (c + 1) * 128],
                        start=(k == 0),
                        stop=(k == n_mm - 1),
                        skip_group_check=True,
                    )
                    k += 1
                if include_dense_bias:
                    mm = e.matmul(
                        gates_ps[:MD, 128: 256],
                        lhsT=onesd_sb[0:1, :],
                        rhs=bd_sb[0:1, :],
                        start=False,
                        stop=True,
                        skip_group_check=True,
                    )
                mm.then_inc(densesem, 1)

            def wh_group(s, t):
                pp = (t - 1) % 2
                for k in range(N_CORES):
                    e.wait_ge(hsems[k][s][pp], 2 * nbc(t - 1))
                for m in range(4):
                    for c in range(8):
                        mm = e.matmul(
                            gtile(s, m),
                            lhsT=wh_sb[:, (c * 4 + m) * 128: (c * 4 + m + 1) * 128],
                            rhs=hslot(s, pp, c),
                            start=False,
                            stop=(c == 7),
                            skip_group_check=True,
                        )
                    if m in (2, 3):
                        mm.then_inc(gsem[s], 1)

            wx_group(0, 0, True)
            wx_group(1, 0, True)
            for t in range(T):
                for s in range(2):
                    if t >= 1:
                        wh_group(s, t)
                if t >= 18 and (t - 18) % dense_blk == 0:
                    dense_block((t - 18) // dense_blk, t)
                for s in range(2):
                    if t + 1 < T:
                        wx_group(s, t + 1, False)
            for bi in range((T - 18) // dense_blk + 1, n_blk):
                dense_block(bi, T - 1)

        @block.scalar
        def _(a: bass.BassScalarEngine):
            def sig_fij(s, t):
                p = t % 2
                a.wait_ge(gsem[s], 2 * t + 1)
                a.activation(
                    sq_cols(s, p, 0, 96).rearrange("p (b v) -> p b v", b=3),
                    gates_r[:, s * 4: s * 4 + 3, 0:SB],
                    AF.Sigmoid,
                ).then_inc(asem[s], 1)

            def sig_o(s, t):
                p = t % 2
                a.wait_ge(gsem[s], 2 * t + 2)
                a.activation(
                    sq_cols(s, p, 96, 128),
                    gtile(s, 3),
                    AF.Sigmoid,
                ).then_inc(asem[s], 1)

            def sig4(s, t):
                p = t % 2
                a.wait_ge(gsem[s], 2 * t + 2)
                a.activation(
                    sq_cols(s, p, 0, 128).rearrange("p (b v) -> p b v", b=4),
                    gates_r[:, s * 4: s * 4 + 4, 0:SB],
                    AF.Sigmoid,
                ).then_inc(asem[s], 1)

            def tc(s, t):
                # tanh(c) as sigmoid(2c)
                a.wait_ge(csem[s], t + 2)
                a.activation(
                    tcb[:, s * SB: (s + 1) * SB],
                    c_sb[:, s * SB: (s + 1) * SB],
                    AF.Sigmoid,
                    scale=2.0,
                ).then_inc(a2sem[s], 1)

            for t in range(T):
                if act_order == "sig4":
                    sig4(0, t), sig4(1, t), tc(0, t), tc(1, t)
                elif tc_slot == "own":
                    sig_fij(0, t), sig_o(0, t), tc(0, t)
                    sig_fij(1, t), sig_o(1, t), tc(1, t)
                elif tc_slot == "split":
                    sig_fij(0, t), sig_o(0, t), sig_fij(1, t), sig_o(1, t)
                    tc(0, t), tc(1, t)
                elif tc_slot == "lateB":
                    sig_fij(0, t)
                    if t >= 1:
                        tc(1, t - 1)
                    sig_o(0, t), tc(0, t), sig_fij(1, t), sig_o(1, t)
            if tc_slot == "lateB" and act_order != "sig4":
                tc(1, T - 1)

        hp_count = {}

        @block.vector
        def _(v: bass.BassVectorEngine):
            v.memset(ones_sb[:, :], 1.0).then_inc(constsem, 1)
            if include_dense_bias:
                v.memset(onesd_sb[:, :], 1.0).then_inc(constsem, 1)
            for s in range(2):
                v.memset(c_sb[:, s * SB: (s + 1) * SB], 0.0).then_inc(csem[s], 1)

            def ostage_copy(bi):
                v.wait_ge(densesem, bi + 1)
                if bi >= 2:
                    v.wait_ge(outdma, 16 * (bi - 1))
                v.tensor_scalar_add(
                    ostage[:MD, (bi % 2) * 128: (bi % 2) * 128 + 128],
                    gates_ps[:MD, 128: 256], 0.0).then_inc(outcp, 1)

            nv = [0]

            def cell(s, t):
                # cf = sf*c ; v = (sj-0.5)*si ; c = 2v+cf
                p = t % 2
                cs = c_sb[:, s * SB: (s + 1) * SB]
                v.wait_ge(asem[s], n_asem_step * t + 1)
                v.wait_ge(csem[s], t + 1)
                nv[0] += 1
                v.tensor_tensor(
                    cf_sb[:, s * SB: (s + 1) * SB],
                    sq_cols(s, p, 0, SB), cs, AluOpType.mult).then_inc(vsem, 1)
                nv[0] += 1
                v.scalar_tensor_tensor(
                    v_sb[:, s * SB: (s + 1) * SB],
                    sq_cols(s, p, 64, 96), 0.5, sq_cols(s, p, SB, 64),
                    AluOpType.subtract, AluOpType.mult).then_inc(vsem, 1)
                v.wait_ge(vsem, nv[0])
                v.scalar_tensor_tensor(
                    cs,
                    v_sb[:, s * SB: (s + 1) * SB], 2.0,
                    cf_sb[:, s * SB: (s + 1) * SB],
                    AluOpType.mult, AluOpType.add).then_inc(csem[s], 1)

            def hprime(s, t):
                # h' = (tanh(c)/2)*sigmoid(o) = (tcb-0.5)*so
                p = t % 2
                if t >= 1:
                    v.wait_ge(lsems[s], 16 * t)
                v.wait_ge(a2sem[s], t + 1)
                nv[0] += 1
                hp_count[(s, t)] = nv[0]
                v.scalar_tensor_tensor(
                    hst(s, p),
                    tcb[:, s * SB: (s + 1) * SB], 0.5,
                    sq_cols(s, p, 96, 128),
                    AluOpType.subtract, AluOpType.mult).then_inc(vsem, 1)

            for t in range(T):
                if t >= 1:
                    hprime(1, t - 1)
                cell(0, t)
                if dve_hp == "early":
                    hprime(0, t)
                    cell(1, t)
                else:
                    cell(1, t)
                    hprime(0, t)
                if t >= 19 and (t - 19) % dense_blk == 0:
                    ostage_copy((t - 19) // dense_blk)
            hprime(1, T - 1)
            for bi in range((T - 19) // dense_blk + 1, n_blk):
                ostage_copy(bi)

        @block.gpsimd
        def _(g: bass.BassGpSimd):
            myg = g.partition_id()
            hbr = hbuf[:, :].rearrange("p (u c v) -> p u c v", u=4, c=8)
            hist_r = hist[:, :].rearrange("p (q c w) -> p q c w", q=2, c=8)

            def hist_copy(k, tau):
                # copy own 8 batch rows of step tau into hist
                s = k // 4
                off = (k % 4) * 8
                pt = tau % 2
                for j in range(N_CORES):
                    g.wait_ge(hsems[j][s][pt], 2 * nbc(tau))
                blk = tau // dense_blk
                tl = tau % dense_blk
                if tl == 0 and blk >= 2:
                    g.wait_ge(densesem, blk - 1)
                g.tensor_copy(
                    hist_r[:, blk % 2, :, tl * 8: tl * 8 + 8],
                    hbr[:, s * 2 + pt, :, off: off + 8],
                ).then_inc(histsem, 1)

            for k in g.Switch(myg, N_CORES):
                s_mine = k // 4
                for t in range(T):
                    p = t % 2
                    for s in range(2):
                        g.remote_dma_broadcast(
                            out_ap=hslot(s, p, k),
                            in_ap=hst(s, p),
                            remote_sem=hsems[k][s][p],
                            local_sem=lsems[s],
                            rdests=[(0, d) for d in range(N_CORES)],
                        ).then_inc(prep, 1)
                        if s == s_mine and t >= 1:
                            hist_copy(k, t - 1)
                            g.wait_ge(histsem, t)
                        g.wait_ge(prep, 2 * t + s + 1)
                        g.wait_ge(vsem, hp_count[(s, t)])
                        g.trigger_dma(count=1)
                hist_copy(k, T - 1)

    nc.finalize()
    return nc


_BUILD_CACHE = {}


def prep_inputs(X, Wx, Wh, b, Wd, bd):
    X = np.asarray(X, dtype=np.float32)
    Wx = np.asarray(Wx, dtype=np.float32)
    Wh = np.asarray(Wh, dtype=np.float32)
    b = np.asarray(b, dtype=np.float32)
    Wd = np.asarray(Wd, dtype=np.float32)
    bd = np.asarray(bd, dtype=np.float32)
    Bsz, T, _ = X.shape
    assert Bsz == B
    include_dense_bias = bool(np.any(bd))

    bf = ml_dtypes.bfloat16
    XT = np.ascontiguousarray(np.transpose(X, (2, 1, 0))).reshape(128, T * Bsz)
    XT = XT.astype(bf)
    Wd2 = (2.0 * Wd).astype(bf)
    in_maps = []
    for k in range(N_CORES):
        cols = []
        for gate in (2, 0, 1, 3):  # tiles f, i, j, o from reference order i,j,f,o
            lo = gate * H + k * HC
            cols.append(np.arange(lo, lo + HC))
        cols = np.concatenate(cols)
        # per-tile gate scale: j tile (tile index 2) doubled (tanh->sigmoid)
        gscale = np.ones(G, np.float32)
        gscale[2 * HC: 3 * HC] = 2.0
        wx = Wx[:, cols] * gscale[None, :]
        wh = Wh[:, cols] * (2.0 * gscale)[None, :]
        brow = b[cols] * gscale
        brow[0:HC] += 1.0  # forget bias
        m = {
            "XT": XT,
            "WX": np.ascontiguousarray(wx).astype(bf),
            "WH": np.ascontiguousarray(wh).astype(bf),
            "WD": Wd2,
            "BROW": np.ascontiguousarray(brow)[None, :].astype(bf),
        }
        if include_dense_bias:
            m["BD"] = np.ascontiguousarray(bd)[None, :].astype(bf)
        in_maps.append(m)
    return in_maps


def assemble_output(results, T):
    outs = []
    for k in range(N_CORES):
        o = np.asarray(results[k]["OUT"]).reshape(T, 8, 128).transpose(1, 0, 2)
        outs.append(o)
    return np.concatenate(outs, axis=0).astype(np.float32)


def kernel(X, Wx, Wh, b, Wd, bd):
    b = np.asarray(b, dtype=np.float32)
    bd = np.asarray(bd, dtype=np.float32)
    T = np.asarray(X).shape[1]
    dense_blk = 16
    include_bias = bool(np.any(b))
    include_dense_bias = bool(np.any(bd))

    key = (T, dense_blk, include_bias, include_dense_bias)
    if key not in _BUILD_CACHE:
        _BUILD_CACHE[key] = build(T, dense_blk, include_bias, include_dense_bias)
    nc = _BUILD_CACHE[key]

    in_maps = prep_inputs(X, Wx, Wh, b, Wd, bd)

    res = None
    for attempt in range(3):
        try:
            res = run_bass_kernel_spmd(nc, in_maps, core_ids=list(range(N_CORES)))
            break
        except Exception:
            if attempt == 2:
                raise

    return assemble_output([res.results[k] for k in range(N_CORES)], T)
